# revision 1
# baseline (speedup 1.0000x reference)
"""Trainium2 Bass kernel for nn_CombineNode_7395933684091 (gnn_message_passing).

Hierarchy: 128 leaf terms (each D=1024 -> H=32), 16 internal terms
(concat of 8 children hiddens, 256 -> 32), 1 root (concat of 16
internal hiddens, 512 -> 32); every term also has a 1-dim predict head.
All matmuls followed by tanh.

Strategy: data-parallel over batch across 8 cores (Bc = 1024 rows per
core), weights replicated. On-chip layout keeps hidden features on the
PARTITION axis ("h^T layout": tiles are [features, batch]), so every
level's contraction is a natural PE matmul and the child-concat is just
stacking partition tiles. x and all weights are repacked on the host so
every DMA is contiguous per partition.

Leaf level: 4 panels x 8 groups (4 leaves) x 8 k-chunk accumulated
[128,128]x[128,512] matmuls. The per-term predict heads ride along as
extra block-diagonal columns fused into the internal-level stationary
operand (cw) and the root-level stationary operand (rw2), so they cost
no extra PE streaming.

Matmuls stream as float32r (full-rate fp32 PE mode, ~tf32 rounding;
plain float32 runs 4x slower at full precision). f32r matmuls must
write PSUM at partition offset 0, hence the one-bank-per-node layout.
"""

import numpy as np

B, D, H = 8192, 1024, 32
L, I, CPI = 128, 16, 8
NCORES = 8
BC = B // NCORES      # 1024 batch rows per core
BN = 512              # batch tile width (one PSUM bank of f32)
NBH = BC // BN        # 2 batch halves
KC = D // 128         # 8 contraction chunks for the leaf level
NPANEL = 4            # leaf panels (8 groups of 4 leaves each)
GPP = 8               # groups per panel
NOUT = L + I + 1      # 145

MM_DT = "float32r"

_CACHE = {}


def _build_nc():
    from contextlib import ExitStack

    import concourse.mybir as mybir
    import concourse.tile as tile
    from concourse import bacc

    f32 = mybir.dt.float32
    Tanh = mybir.ActivationFunctionType.Tanh
    mmdt = getattr(mybir.dt, MM_DT)

    nc = bacc.Bacc("TRN2", target_bir_lowering=False, debug=False)

    xt = nc.dram_tensor("xt", [D, BC], mmdt, kind="ExternalInput")
    lw = nc.dram_tensor("lw", [D, L * H], mmdt, kind="ExternalInput")
    lb = nc.dram_tensor("lb", [128, 32], f32, kind="ExternalInput")
    # fused internal-trans + leaf-predict stationary: per (node i, chunk j)
    # a [128, 64] block: cols 0:32 int_W chunk, cols 32+4j+c leaf Wp diag
    cw = nc.dram_tensor("cw", [128, I * 2 * 64], mmdt, kind="ExternalInput")
    intb = nc.dram_tensor("intb", [128, 4], f32, kind="ExternalInput")
    lbp8 = nc.dram_tensor("lbp8", [8, 16], f32, kind="ExternalInput")
    # fused root-trans + int-predict stationary: per panel q a [128, 48]
    # block: cols 0:32 root_W chunk, cols 32:48 int Wp diag
    rw2 = nc.dram_tensor("rw2", [128, NPANEL * 48], mmdt, kind="ExternalInput")
    intbp = nc.dram_tensor("intbp", [16, 1], f32, kind="ExternalInput")
    rootb = nc.dram_tensor("rootb", [32, 1], f32, kind="ExternalInput")
    rootwp = nc.dram_tensor("rootwp", [32, 1], mmdt, kind="ExternalInput")
    rootbp = nc.dram_tensor("rootbp", [1, 1], f32, kind="ExternalInput")
    out = nc.dram_tensor("out", [NOUT, BC], f32, kind="ExternalOutput")

    mm = nc.tensor.matmul

    with tile.TileContext(nc) as tc, ExitStack() as ctx:
        consts = ctx.enter_context(tc.tile_pool(name="consts", bufs=1))
        wpool = ctx.enter_context(tc.tile_pool(name="wpool", bufs=3))
        work = ctx.enter_context(tc.tile_pool(name="work", bufs=4))
        keep = ctx.enter_context(tc.tile_pool(name="keep", bufs=1))
        psum = ctx.enter_context(tc.tile_pool(name="psum", bufs=1, space="PSUM"))

        # --- PE pre-warm: ~4us of dummy matmuls unthrottles the HAM clock
        # gate (PE boots at 1.2 GHz; 3.4us of sustained activity -> 2.4 GHz).
        # Uses a preloaded const AP so nothing gates the first matmul.
        warm_c = nc.const_aps.tensor(0.0, (128, 64), f32)
        pwarm = psum.tile([64, 64], f32, tag="misc", bufs=1, name="pwarm")
        # 18 f32 calls = 36 split-MMs x ~107ns cold ~= 3.9us: enough to trip
        # the HAM busy window, short enough to drain before real data lands
        for _ in range(22):
            mm(pwarm[:], warm_c, warm_c, start=True, stop=True,
               skip_group_check=True)

        # --- loads, ordered so panel-0 compute overlaps the DMA preamble:
        # (xt bn0 | wp0 cols 0:512) -> lb,cw -> xt bn1 -> wp0 cols 512:1024
        xt_sb = consts.tile([128, KC * BC], mmdt, name="xt_sb")
        wp0 = wpool.tile([128, KC * 1024], mmdt, tag="wpanel", name="wp0")
        for k in range(KC):
            nc.sync.dma_start(
                xt_sb[:, k * BC:k * BC + BN], xt[k * 128:(k + 1) * 128, 0:BN]
            )
            nc.sync.dma_start(
                wp0[:, k * 1024:k * 1024 + 512], lw[k * 128:(k + 1) * 128, 0:512]
            )
        lb_sb = consts.tile([128, 32], f32, name="lb_sb")
        nc.sync.dma_start(lb_sb[:], lb[:])
        intb_sb = consts.tile([128, 4], f32, name="intb_sb")
        nc.sync.dma_start(intb_sb[:], intb[:])
        lbp8_sb = consts.tile([8, 16], f32, name="lbp8_sb")
        nc.sync.dma_start(lbp8_sb[:], lbp8[:])
        for k in range(KC):
            nc.sync.dma_start(
                wp0[:, k * 1024 + 512:(k + 1) * 1024],
                lw[k * 128:(k + 1) * 128, 512:1024],
            )
        cw_sb = consts.tile([128, I * 2 * 64], mmdt, name="cw_sb")
        nc.sync.dma_start(cw_sb[:], cw[:])
        for k in range(KC):
            nc.sync.dma_start(
                xt_sb[:, k * BC + BN:(k + 1) * BC],
                xt[k * 128:(k + 1) * 128, BN:BC],
            )
        rw2_sb = consts.tile([128, NPANEL * 48], mmdt, name="rw2_sb")
        nc.sync.dma_start(rw2_sb[:], rw2[:])
        intbp_sb = consts.tile([16, 1], f32, name="intbp_sb")
        nc.sync.dma_start(intbp_sb[:], intbp[:])
        rootb_sb = consts.tile([32, 1], f32, name="rootb_sb")
        nc.sync.dma_start(rootb_sb[:], rootb[:])
        rootwp_sb = consts.tile([32, 1], mmdt, name="rootwp_sb")
        nc.sync.dma_start(rootwp_sb[:], rootwp[:])
        rootbp_sb = consts.tile([1, 1], f32, name="rootbp_sb")
        nc.sync.dma_start(rootbp_sb[:], rootbp[:])

        intp_sb = keep.tile([16, BC], f32, name="intp_sb")
        rootp_sb = keep.tile([1, BC], f32, name="rootp_sb")

        inth = {}  # (panel, bn) -> [128, BN] tile: int nodes 4p..4p+3 h^T

        # wp1/wp2 loads emitted up front: SP issues them right after the
        # preamble instead of FIFO-blocking behind panel-0's output stores
        wps = {0: wp0}
        for q in (1, 2):
            wps[q] = wpool.tile([128, KC * 1024], mmdt, tag="wpanel", name=f"wp{q}")
            for k in range(KC):
                nc.sync.dma_start(
                    wps[q][:, k * 1024:(k + 1) * 1024],
                    lw[k * 128:(k + 1) * 128, q * 1024:(q + 1) * 1024],
                )

        # --- leaf + internal levels ----------------------------------------
        for p in range(NPANEL):
            if p in wps:
                wp = wps[p]
            else:
                wp = wpool.tile([128, KC * 1024], mmdt, tag="wpanel", name=f"wp{p}")
                for k in range(KC):
                    nc.sync.dma_start(
                        wp[:, k * 1024:(k + 1) * 1024],
                        lw[k * 128:(k + 1) * 128, p * 1024:(p + 1) * 1024],
                    )
            for bn in range(NBH):
                ith = keep.tile([128, BN], mmdt, tag=f"inth{p}{bn}", name=f"inth{p}{bn}")

                def comb_mm(il, j, lh, pcomb):
                    """Fused internal-trans + leaf-predict matmul.

                    pcomb rows 0:32 accumulate node (4p+il)'s hidden
                    pre-activation over its two child groups; rows 32:40
                    pick up the group's 4 leaf predict dots via the
                    block-diagonal columns (zeros elsewhere)."""
                    i = 4 * p + il
                    mm(
                        pcomb[:],
                        cw_sb[:, (2 * i + j) * 64:(2 * i + j + 1) * 64],
                        lh[:],
                        start=(j == 0),
                        stop=(j == 1),
                        skip_group_check=True,
                    )

                def comb_post(il, pcomb):
                    i = 4 * p + il
                    nc.scalar.activation(
                        ith[32 * il:32 * il + 32, :],
                        pcomb[0:32, :],
                        Tanh,
                        bias=intb_sb[32 * il:32 * il + 32, p:p + 1],
                    )
                    lptmp = work.tile([8, BN], f32, tag="lp", name=f"lp{i}{bn}")
                    nc.scalar.activation(
                        lptmp[:], pcomb[32:40, :], Tanh, bias=lbp8_sb[:, i:i + 1]
                    )
                    nc.gpsimd.dma_start(
                        out[8 * i:8 * i + 8, bn * BN:bn * BN + BN], lptmp[:]
                    )

                def leaf_mm(gl, k, pg):
                    mm(
                        pg[:],
                        wp[:, k * 1024 + gl * 128:k * 1024 + (gl + 1) * 128],
                        xt_sb[:, k * BC + bn * BN:k * BC + bn * BN + BN],
                        start=(k == 0),
                        stop=(k == KC - 1),
                    )

                def leaf_tanh(gl, pg):
                    lh = work.tile([128, BN], mmdt, tag="lh", name=f"lh{p}{bn}{gl}")
                    nc.scalar.activation(
                        lh[:], pg[:], Tanh, bias=lb_sb[:, GPP * p + gl:GPP * p + gl + 1]
                    )
                    return lh

                if p == 0:
                    # k-outer over a 5-group then 3-group wave: matmuls start
                    # as soon as the first xt/wp chunks land, and the first
                    # wave keeps 5 matmuls in flight per arriving chunk
                    pend = {}
                    for g0, cnt in ((0, 5), (5, 3)):
                        pgs = [
                            psum.tile([128, BN], f32, tag="pg", bufs=5,
                                      name=f"pgko{bn}{g0}{q}")
                            for q in range(cnt)
                        ]
                        for k in range(KC):
                            for q in range(cnt):
                                leaf_mm(g0 + q, k, pgs[q])
                        for q in range(cnt):
                            gl = g0 + q
                            il, j = divmod(gl, 2)
                            if j == 0:
                                pend[il] = psum.tile([64, BN], f32, tag="pcomb",
                                                     bufs=2, name=f"pcko{bn}{il}")
                            lh = leaf_tanh(gl, pgs[q])
                            comb_mm(il, j, lh, pend[il])
                            if j == 1:
                                comb_post(il, pend.pop(il))
                else:
                    for il in range(4):
                        pcomb = psum.tile([64, BN], f32, tag="pcomb", bufs=2,
                                          name=f"pc{p}{bn}{il}")
                        for j in range(2):
                            gl = 2 * il + j
                            pg = psum.tile([128, BN], f32, tag="pg", bufs=5,
                                           name=f"pg{p}{bn}{gl}")
                            for k in range(KC):
                                leaf_mm(gl, k, pg)
                            lh = leaf_tanh(gl, pg)
                            comb_mm(il, j, lh, pcomb)
                        comb_post(il, pcomb)
                inth[(p, bn)] = ith

                if p == NPANEL - 1:
                    # fused int-predict + root for this batch half, emitted
                    # here so bn=0's tail overlaps bn=1's leaf stream
                    prc = psum.tile([48, BN], f32, tag="misc", bufs=1, name=f"prc{bn}")
                    for q in range(NPANEL):
                        mm(
                            prc[:],
                            rw2_sb[:, 48 * q:48 * (q + 1)],
                            inth[(q, bn)][:],
                            start=(q == 0),
                            stop=(q == NPANEL - 1),
                            skip_group_check=True,
                        )
                    rh = work.tile([32, BN], mmdt, tag="rh", name=f"rh{bn}")
                    nc.scalar.activation(rh[:], prc[0:32, :], Tanh,
                                         bias=rootb_sb[:, 0:1])
                    nc.scalar.activation(
                        intp_sb[:, bn * BN:bn * BN + BN], prc[32:48, :], Tanh,
                        bias=intbp_sb[:, 0:1],
                    )
                    nc.sync.dma_start(
                        out[L:L + I, bn * BN:bn * BN + BN],
                        intp_sb[:, bn * BN:bn * BN + BN],
                    )
                    prp = psum.tile([1, BN], f32, tag="misc", bufs=1, name=f"prp{bn}")
                    mm(prp[:], rootwp_sb[:], rh[:], start=True, stop=True)
                    nc.scalar.activation(
                        rootp_sb[0:1, bn * BN:bn * BN + BN], prp[:], Tanh,
                        bias=rootbp_sb[:, 0:1],
                    )
                    nc.sync.dma_start(
                        out[L + I:NOUT, bn * BN:bn * BN + BN],
                        rootp_sb[0:1, bn * BN:bn * BN + BN],
                    )

    nc.compile()
    return nc


def _pack_weights(inp):
    f = np.float32
    leaf_b = np.asarray(inp["leaf_b"], f)
    int_W = np.asarray(inp["int_W"], f)
    int_b = np.asarray(inp["int_b"], f)
    root_W = np.asarray(inp["root_W"], f)
    root_b = np.asarray(inp["root_b"], f)
    leaf_Wp = np.asarray(inp["leaf_Wp"], f)
    leaf_bp = np.asarray(inp["leaf_bp"], f)
    int_Wp = np.asarray(inp["int_Wp"], f)
    int_bp = np.asarray(inp["int_bp"], f)
    root_Wp = np.asarray(inp["root_Wp"], f)
    root_bp = np.asarray(inp["root_bp"], f)

    w = {}
    w["lw"] = np.ascontiguousarray(
        np.asarray(inp["leaf_W"], f).transpose(1, 0, 2).reshape(D, L * H)
    )
    w["lb"] = np.ascontiguousarray(leaf_b.reshape(32, 128).T)

    cw = np.zeros((128, I * 2 * 64), f)
    for i in range(I):
        for j in range(2):
            base = (2 * i + j) * 64
            # int_W chunk j of node i: rows (c*32+h) = child (4j+c) hidden h
            cw[:, base:base + 32] = int_W[i, 128 * j:128 * (j + 1), :]
            for c in range(4):
                lv = 8 * i + 4 * j + c
                cw[c * 32:(c + 1) * 32, base + 32 + 4 * j + c] = leaf_Wp[lv, :, 0]
    w["cw"] = cw
    w["intb"] = np.ascontiguousarray(int_b.reshape(4, 128).T)
    w["lbp8"] = np.ascontiguousarray(leaf_bp.reshape(16, 8).T)

    rw2 = np.zeros((128, NPANEL * 48), f)
    for q in range(NPANEL):
        rw2[:, 48 * q:48 * q + 32] = root_W[128 * q:128 * (q + 1), :]
        for c in range(4):
            iv = 4 * q + c
            rw2[c * 32:(c + 1) * 32, 48 * q + 32 + 4 * q + c] = int_Wp[iv, :, 0]
    w["rw2"] = rw2
    w["intbp"] = np.ascontiguousarray(int_bp.reshape(16, 1))
    w["rootb"] = np.ascontiguousarray(root_b.reshape(32, 1))
    w["rootwp"] = np.ascontiguousarray(root_Wp.reshape(32, 1))
    w["rootbp"] = np.ascontiguousarray(root_bp.reshape(1, 1))
    return w


def kernel(**inputs):
    from concourse.bass_utils import run_bass_kernel_spmd

    nc = _CACHE.get("nc")
    if nc is None:
        nc = _CACHE["nc"] = _build_nc()

    x = np.asarray(inputs["x"], np.float32)
    w = _pack_weights(inputs)
    in_maps = []
    for c in range(NCORES):
        m = dict(w)
        m["xt"] = np.ascontiguousarray(x[c * BC:(c + 1) * BC, :].T)
        in_maps.append(m)

    res = run_bass_kernel_spmd(nc, in_maps, core_ids=list(range(NCORES)))
    _CACHE["last_res"] = res
    outs = [res.results[c]["out"] for c in range(NCORES)]
    full = np.concatenate([o[:, :, None] for o in outs], axis=1)  # [145, B, 1]
    return full.astype(np.float32)



# revision 9
# speedup vs baseline: 1.0045x; 1.0045x over previous
"""Trainium2 Bass kernel for nn_CombineNode_7395933684091 (gnn_message_passing).

Hierarchy: 128 leaf terms (each D=1024 -> H=32), 16 internal terms
(concat of 8 children hiddens, 256 -> 32), 1 root (concat of 16
internal hiddens, 512 -> 32); every term also has a 1-dim predict head.
All matmuls followed by tanh.

Strategy: data-parallel over batch across 8 cores (Bc = 1024 rows per
core), weights replicated. On-chip layout keeps hidden features on the
PARTITION axis ("h^T layout": tiles are [features, batch]), so every
level's contraction is a natural PE matmul and the child-concat is just
stacking partition tiles. x and all weights are repacked on the host so
every DMA is contiguous per partition.

Leaf level: 4 panels x 8 groups (4 leaves) x 8 k-chunk accumulated
[128,128]x[128,512] matmuls. The per-term predict heads ride along as
extra block-diagonal columns fused into the internal-level stationary
operand (cw) and the root-level stationary operand (rw2), so they cost
no extra PE streaming.

Matmul operands are float16: same PE stream rate as f32r (1 col/cycle)
but enables Fast Weight Load (fp32 disables FWL) so LDWEIGHTS hides
behind the matmul stream, and halves HBM + SBUF traffic. fp16's 10
mantissa bits keep the end-to-end max abs error ~1.6e-3 (vs 2e-2 gate).
Predict outputs are accumulated into per-batch-half SBUF tiles and
stored with one DMA each (fewer queues -> shorter drain tail).
"""

import numpy as np

B, D, H = 8192, 1024, 32
L, I, CPI = 128, 16, 8
NCORES = 8
BC = B // NCORES      # 1024 batch rows per core
BN = 512              # batch tile width (one PSUM bank of f32)
NBH = BC // BN        # 2 batch halves
KC = D // 128         # 8 contraction chunks for the leaf level
NPANEL = 4            # leaf panels (8 groups of 4 leaves each)
GPP = 8               # groups per panel
NOUT = L + I + 1      # 145

MM_DT = "float16"

_CACHE = {}


def _build_nc():
    from contextlib import ExitStack

    import concourse.mybir as mybir
    import concourse.tile as tile
    from concourse import bacc

    f32 = mybir.dt.float32
    Tanh = mybir.ActivationFunctionType.Tanh
    mmdt = getattr(mybir.dt, MM_DT)

    nc = bacc.Bacc("TRN2", target_bir_lowering=False, debug=False)

    xt = nc.dram_tensor("xt", [D, BC], mmdt, kind="ExternalInput")
    lw = nc.dram_tensor("lw", [D, L * H], mmdt, kind="ExternalInput")
    lb = nc.dram_tensor("lb", [128, 32], f32, kind="ExternalInput")
    # fused internal-trans + leaf-predict stationary: per (node i, chunk j)
    # a [128, 64] block: cols 0:32 int_W chunk, cols 32+4j+c leaf Wp diag
    cw = nc.dram_tensor("cw", [128, I * 2 * 64], mmdt, kind="ExternalInput")
    intb = nc.dram_tensor("intb", [128, 4], f32, kind="ExternalInput")
    lbp8 = nc.dram_tensor("lbp8", [8, 16], f32, kind="ExternalInput")
    # fused root-trans + int-predict stationary: per panel q a [128, 48]
    # block: cols 0:32 root_W chunk, cols 32:48 int Wp diag
    rw2 = nc.dram_tensor("rw2", [128, NPANEL * 48], mmdt, kind="ExternalInput")
    intbp = nc.dram_tensor("intbp", [16, 1], f32, kind="ExternalInput")
    rootb = nc.dram_tensor("rootb", [32, 1], f32, kind="ExternalInput")
    rootwp = nc.dram_tensor("rootwp", [32, 1], mmdt, kind="ExternalInput")
    rootbp = nc.dram_tensor("rootbp", [1, 1], f32, kind="ExternalInput")
    out = nc.dram_tensor("out", [NOUT, BC], f32, kind="ExternalOutput")

    mm = nc.tensor.matmul

    with tile.TileContext(nc) as tc, ExitStack() as ctx:
        consts = ctx.enter_context(tc.tile_pool(name="consts", bufs=1))
        wpool = ctx.enter_context(tc.tile_pool(name="wpool", bufs=4))
        work = ctx.enter_context(tc.tile_pool(name="work", bufs=4))
        keep = ctx.enter_context(tc.tile_pool(name="keep", bufs=1))
        psum = ctx.enter_context(tc.tile_pool(name="psum", bufs=1, space="PSUM"))

        # --- PE pre-warm: ~4us of dummy matmuls unthrottles the HAM clock
        # gate (PE boots at 1.2 GHz; 3.4us of sustained activity -> 2.4 GHz).
        # Uses a preloaded const AP so nothing gates the first matmul.
        warm_c = nc.const_aps.tensor(0.0, (128, 64), f32)
        pwarm = psum.tile([64, 64], f32, tag="misc", bufs=1, name="pwarm")
        # f32 calls run as LOW+HIGH split-MM pairs: enough sustained PE
        # activity to trip the HAM busy window before real data lands
        for _ in range(22):
            mm(pwarm[:], warm_c, warm_c, start=True, stop=True,
               skip_group_check=True)

        # --- loads, ordered so panel-0 compute overlaps the DMA preamble:
        # (xt bn0 | wp0 cols 0:512) -> lb,cw -> xt bn1 -> wp0 cols 512:1024
        xt_sb = consts.tile([128, KC * BC], mmdt, name="xt_sb")
        wp0 = wpool.tile([128, KC * 1024], mmdt, tag="wpanel", name="wp0")
        for k in range(KC):
            nc.sync.dma_start(
                xt_sb[:, k * BC:k * BC + BN], xt[k * 128:(k + 1) * 128, 0:BN]
            )
            nc.sync.dma_start(
                wp0[:, k * 1024:k * 1024 + 512], lw[k * 128:(k + 1) * 128, 0:512]
            )
        lb_sb = consts.tile([128, 32], f32, name="lb_sb")
        nc.sync.dma_start(lb_sb[:], lb[:])
        intb_sb = consts.tile([128, 4], f32, name="intb_sb")
        nc.sync.dma_start(intb_sb[:], intb[:])
        lbp8_sb = consts.tile([8, 16], f32, name="lbp8_sb")
        nc.sync.dma_start(lbp8_sb[:], lbp8[:])
        for k in range(KC):
            nc.sync.dma_start(
                wp0[:, k * 1024 + 512:(k + 1) * 1024],
                lw[k * 128:(k + 1) * 128, 512:1024],
            )
        cw_sb = consts.tile([128, I * 2 * 64], mmdt, name="cw_sb")
        nc.sync.dma_start(cw_sb[:], cw[:])
        for k in range(KC):
            nc.sync.dma_start(
                xt_sb[:, k * BC + BN:(k + 1) * BC],
                xt[k * 128:(k + 1) * 128, BN:BC],
            )
        rw2_sb = consts.tile([128, NPANEL * 48], mmdt, name="rw2_sb")
        nc.sync.dma_start(rw2_sb[:], rw2[:])
        intbp_sb = consts.tile([16, 1], f32, name="intbp_sb")
        nc.sync.dma_start(intbp_sb[:], intbp[:])
        rootb_sb = consts.tile([32, 1], f32, name="rootb_sb")
        nc.sync.dma_start(rootb_sb[:], rootb[:])
        rootwp_sb = consts.tile([32, 1], mmdt, name="rootwp_sb")
        nc.sync.dma_start(rootwp_sb[:], rootwp[:])
        rootbp_sb = consts.tile([1, 1], f32, name="rootbp_sb")
        nc.sync.dma_start(rootbp_sb[:], rootbp[:])

        # per-node leaf-predict staging: [8, BC] tiles (both batch halves)
        # flushed with one DMA per node after bn=1 (halves the store count
        # vs per-half stores; engine APs need 32-aligned partition bases so
        # denser packing is not possible)
        lp_node = {}
        intp_sb = keep.tile([16, BC], f32, name="intp_sb")
        rootp_sb = keep.tile([1, BC], f32, name="rootp_sb")

        inth = {}  # (panel, bn) -> [128, BN] tile: int nodes 4p..4p+3 h^T

        # all remaining panel loads emitted up front: SP issues them right
        # after the preamble; fp16 panels are small enough to coexist
        wps = {0: wp0}
        for q in (1, 2, 3):
            wps[q] = wpool.tile([128, KC * 1024], mmdt, tag="wpanel", name=f"wp{q}")
            for k in range(KC):
                nc.sync.dma_start(
                    wps[q][:, k * 1024:(k + 1) * 1024],
                    lw[k * 128:(k + 1) * 128, q * 1024:(q + 1) * 1024],
                )

        # --- leaf + internal levels ----------------------------------------
        for p in range(NPANEL):
            wp = wps[p]
            for bn in range(NBH):
                ith = keep.tile([128, BN], mmdt, tag=f"inth{p}{bn}", name=f"inth{p}{bn}")

                def comb_mm(il, j, lh, pcomb):
                    """Fused internal-trans + leaf-predict matmul.

                    pcomb rows 0:32 accumulate node (4p+il)'s hidden
                    pre-activation over its two child groups; rows 32:40
                    pick up the group's 4 leaf predict dots via the
                    block-diagonal columns (zeros elsewhere)."""
                    i = 4 * p + il
                    mm(
                        pcomb[:],
                        cw_sb[:, (2 * i + j) * 64:(2 * i + j + 1) * 64],
                        lh[:],
                        start=(j == 0),
                        stop=(j == 1),
                        skip_group_check=True,
                    )

                def comb_post(il, pcomb):
                    i = 4 * p + il
                    nc.scalar.activation(
                        ith[32 * il:32 * il + 32, :],
                        pcomb[0:32, :],
                        Tanh,
                        bias=intb_sb[32 * il:32 * il + 32, p:p + 1],
                    )
                    if i not in lp_node:
                        lp_node[i] = keep.tile([8, BC], f32, name=f"lp{i}")
                    nc.scalar.activation(
                        lp_node[i][:, bn * BN:bn * BN + BN], pcomb[32:40, :],
                        Tanh, bias=lbp8_sb[:, i:i + 1],
                    )
                    if bn == NBH - 1:
                        nc.gpsimd.dma_start(
                            out[8 * i:8 * i + 8, :], lp_node[i][:]
                        )

                def leaf_mm(gl, k, pg):
                    mm(
                        pg[:],
                        wp[:, k * 1024 + gl * 128:k * 1024 + (gl + 1) * 128],
                        xt_sb[:, k * BC + bn * BN:k * BC + bn * BN + BN],
                        start=(k == 0),
                        stop=(k == KC - 1),
                    )

                def leaf_tanh(gl, pg):
                    lh = work.tile([128, BN], mmdt, tag="lh", name=f"lh{p}{bn}{gl}")
                    nc.scalar.activation(
                        lh[:], pg[:], Tanh, bias=lb_sb[:, GPP * p + gl:GPP * p + gl + 1]
                    )
                    return lh

                if p == 0:
                    # k-outer over a 5-group then 3-group wave: matmuls start
                    # as soon as the first xt/wp chunks land, and the first
                    # wave keeps 5 matmuls in flight per arriving chunk
                    pend = {}
                    for g0, cnt in ((0, 5), (5, 3)):
                        pgs = [
                            psum.tile([128, BN], f32, tag="pg", bufs=5,
                                      name=f"pgko{bn}{g0}{q}")
                            for q in range(cnt)
                        ]
                        for k in range(KC):
                            for q in range(cnt):
                                leaf_mm(g0 + q, k, pgs[q])
                        for q in range(cnt):
                            gl = g0 + q
                            il, j = divmod(gl, 2)
                            if j == 0:
                                pend[il] = psum.tile([64, BN], f32, tag="pcomb",
                                                     bufs=2, name=f"pcko{bn}{il}")
                            lh = leaf_tanh(gl, pgs[q])
                            comb_mm(il, j, lh, pend[il])
                            if j == 1:
                                comb_post(il, pend.pop(il))
                else:
                    for il in range(4):
                        pcomb = psum.tile([64, BN], f32, tag="pcomb", bufs=2,
                                          name=f"pc{p}{bn}{il}")
                        for j in range(2):
                            gl = 2 * il + j
                            pg = psum.tile([128, BN], f32, tag="pg", bufs=5,
                                           name=f"pg{p}{bn}{gl}")
                            for k in range(KC):
                                leaf_mm(gl, k, pg)
                            lh = leaf_tanh(gl, pg)
                            comb_mm(il, j, lh, pcomb)
                        comb_post(il, pcomb)
                inth[(p, bn)] = ith

                if p == NPANEL - 1:
                    # fused int-predict + root for this batch half, emitted
                    # here so bn=0's tail overlaps bn=1's leaf stream
                    prc = psum.tile([48, BN], f32, tag="misc", bufs=1, name=f"prc{bn}")
                    for q in range(NPANEL):
                        mm(
                            prc[:],
                            rw2_sb[:, 48 * q:48 * (q + 1)],
                            inth[(q, bn)][:],
                            start=(q == 0),
                            stop=(q == NPANEL - 1),
                            skip_group_check=True,
                        )
                    rh = work.tile([32, BN], mmdt, tag="rh", name=f"rh{bn}")
                    nc.scalar.activation(rh[:], prc[0:32, :], Tanh,
                                         bias=rootb_sb[:, 0:1])
                    nc.scalar.activation(
                        intp_sb[:, bn * BN:bn * BN + BN], prc[32:48, :], Tanh,
                        bias=intbp_sb[:, 0:1],
                    )
                    prp = psum.tile([1, BN], f32, tag="misc", bufs=1, name=f"prp{bn}")
                    mm(prp[:], rootwp_sb[:], rh[:], start=True, stop=True)
                    nc.scalar.activation(
                        rootp_sb[0:1, bn * BN:bn * BN + BN], prp[:], Tanh,
                        bias=rootbp_sb[:, 0:1],
                    )
                    if bn == NBH - 1:
                        nc.gpsimd.dma_start(out[L:L + I, :], intp_sb[:])
                        nc.gpsimd.dma_start(out[L + I:NOUT, :], rootp_sb[:])

    nc.compile()
    return nc


def _pack_weights(inp):
    f = np.float32
    f16 = np.float16
    leaf_b = np.asarray(inp["leaf_b"], f)
    int_W = np.asarray(inp["int_W"], f)
    int_b = np.asarray(inp["int_b"], f)
    root_W = np.asarray(inp["root_W"], f)
    root_b = np.asarray(inp["root_b"], f)
    leaf_Wp = np.asarray(inp["leaf_Wp"], f)
    leaf_bp = np.asarray(inp["leaf_bp"], f)
    int_Wp = np.asarray(inp["int_Wp"], f)
    int_bp = np.asarray(inp["int_bp"], f)
    root_Wp = np.asarray(inp["root_Wp"], f)
    root_bp = np.asarray(inp["root_bp"], f)

    w = {}
    w["lw"] = np.ascontiguousarray(
        np.asarray(inp["leaf_W"], f16).transpose(1, 0, 2).reshape(D, L * H)
    )
    w["lb"] = np.ascontiguousarray(leaf_b.reshape(32, 128).T)

    cw = np.zeros((128, I * 2 * 64), f16)
    for i in range(I):
        for j in range(2):
            base = (2 * i + j) * 64
            # int_W chunk j of node i: rows (c*32+h) = child (4j+c) hidden h
            cw[:, base:base + 32] = int_W[i, 128 * j:128 * (j + 1), :]
            for c in range(4):
                lv = 8 * i + 4 * j + c
                cw[c * 32:(c + 1) * 32, base + 32 + 4 * j + c] = leaf_Wp[lv, :, 0]
    w["cw"] = cw
    w["intb"] = np.ascontiguousarray(int_b.reshape(4, 128).T)
    w["lbp8"] = np.ascontiguousarray(leaf_bp.reshape(16, 8).T)

    rw2 = np.zeros((128, NPANEL * 48), f16)
    for q in range(NPANEL):
        rw2[:, 48 * q:48 * q + 32] = root_W[128 * q:128 * (q + 1), :]
        for c in range(4):
            iv = 4 * q + c
            rw2[c * 32:(c + 1) * 32, 48 * q + 32 + 4 * q + c] = int_Wp[iv, :, 0]
    w["rw2"] = rw2
    w["intbp"] = np.ascontiguousarray(int_bp.reshape(16, 1))
    w["rootb"] = np.ascontiguousarray(root_b.reshape(32, 1))
    w["rootwp"] = np.ascontiguousarray(root_Wp.reshape(32, 1).astype(f16))
    w["rootbp"] = np.ascontiguousarray(root_bp.reshape(1, 1))
    return w


def kernel(**inputs):
    from concourse.bass_utils import run_bass_kernel_spmd

    nc = _CACHE.get("nc")
    if nc is None:
        nc = _CACHE["nc"] = _build_nc()

    x = np.asarray(inputs["x"], np.float32)
    w = _pack_weights(inputs)
    in_maps = []
    for c in range(NCORES):
        m = dict(w)
        m["xt"] = np.ascontiguousarray(x[c * BC:(c + 1) * BC, :].T.astype(np.float16))
        in_maps.append(m)

    res = run_bass_kernel_spmd(nc, in_maps, core_ids=list(range(NCORES)))
    _CACHE["last_res"] = res
    outs = [res.results[c]["out"] for c in range(NCORES)]
    full = np.concatenate([o[:, :, None] for o in outs], axis=1)  # [145, B, 1]
    return full.astype(np.float32)


# revision 10
# speedup vs baseline: 1.0776x; 1.0728x over previous
"""Trainium2 Bass kernel for nn_CombineNode_7395933684091 (gnn_message_passing).

Hierarchy: 128 leaf terms (each D=1024 -> H=32), 16 internal terms
(concat of 8 children hiddens, 256 -> 32), 1 root (concat of 16
internal hiddens, 512 -> 32); every term also has a 1-dim predict head.
All matmuls followed by tanh.

Strategy: data-parallel over batch across 8 cores (Bc = 1024 rows per
core), weights replicated. On-chip layout keeps hidden features on the
PARTITION axis ("h^T layout": tiles are [features, batch]), so every
level's contraction is a natural PE matmul and the child-concat is just
stacking partition tiles. x and all weights are repacked on the host so
every DMA is contiguous per partition.

Leaf level: 4 panels x 8 groups (4 leaves) x 8 k-chunk accumulated
[128,128]x[128,512] matmuls. The per-term predict heads ride along as
extra block-diagonal columns fused into the internal-level stationary
operand (cw) and the root-level stationary operand (rw2), so they cost
no extra PE streaming.

Matmul operands are float16: same PE stream rate as f32r (1 col/cycle)
but enables Fast Weight Load (fp32 disables FWL) so LDWEIGHTS hides
behind the matmul stream, and halves HBM + SBUF traffic. fp16's 10
mantissa bits keep the end-to-end max abs error ~1.6e-3 (vs 2e-2 gate).

All small stationaries (cw/rw2 blocks) are zero-padded to 128 columns:
narrow (col_grp) matmuls block the next LDWEIGHTS pull-ahead and cost
~+120ns each. The root contraction for the last batch half accumulates
incrementally (one matmul per panel, deferred into the next panel's
stream) so the end-of-kernel serial chain is short.
"""

import numpy as np

B, D, H = 8192, 1024, 32
L, I, CPI = 128, 16, 8
NCORES = 8
BC = B // NCORES      # 1024 batch rows per core
BN = 512              # batch tile width (one PSUM bank of f32)
NBH = BC // BN        # 2 batch halves
KC = D // 128         # 8 contraction chunks for the leaf level
NPANEL = 4            # leaf panels (8 groups of 4 leaves each)
GPP = 8               # groups per panel
NOUT = L + I + 1      # 145

MM_DT = "float16"

_CACHE = {}


def _build_nc():
    from contextlib import ExitStack

    import concourse.mybir as mybir
    import concourse.tile as tile
    from concourse import bacc

    f32 = mybir.dt.float32
    Tanh = mybir.ActivationFunctionType.Tanh
    mmdt = getattr(mybir.dt, MM_DT)

    nc = bacc.Bacc("TRN2", target_bir_lowering=False, debug=False)

    xt = nc.dram_tensor("xt", [D, BC], mmdt, kind="ExternalInput")
    # leaf weights, panel-major: lwh[p, pp, k*1024 + j] so each panel is
    # one contiguous [128, 8K] DMA (16KB/partition runs)
    lwh = nc.dram_tensor("lwh", [NPANEL, 128, KC * 1024], mmdt, kind="ExternalInput")
    # fused internal-trans + leaf-predict stationary: per (node i, chunk j)
    # a [128, 128] block: cols 0:32 int_W chunk, col 32+4j+c leaf Wp diag,
    # rest zero padding (full-width stationaries keep LDW pull-ahead alive)
    cw = nc.dram_tensor("cw", [128, I * 2 * 128], mmdt, kind="ExternalInput")
    # fused root-trans + int-predict stationary: per panel q a [128, 128]
    # block (cols 0:32 root_W chunk, 32:48 int Wp diag, rest zero); final
    # column 512 holds root_Wp in rows 0:32
    rw2 = nc.dram_tensor("rw2", [128, NPANEL * 128 + 1], mmdt, kind="ExternalInput")
    # all f32 per-partition bias constants in one tensor:
    # cols 0:32 leaf_b, 32:36 int_b, 36:52 leaf_bp (rows 0:8),
    # 52 int_bp (rows 0:16), 53 root_b (rows 0:32), 54 root_bp (row 0)
    cc = nc.dram_tensor("cc", [128, 55], f32, kind="ExternalInput")
    out = nc.dram_tensor("out", [NOUT, BC], f32, kind="ExternalOutput")

    mm = nc.tensor.matmul

    with tile.TileContext(nc) as tc, ExitStack() as ctx:
        consts = ctx.enter_context(tc.tile_pool(name="consts", bufs=1))
        wpool = ctx.enter_context(tc.tile_pool(name="wpool", bufs=4))
        work = ctx.enter_context(tc.tile_pool(name="work", bufs=18))
        keep = ctx.enter_context(tc.tile_pool(name="keep", bufs=1))
        psum = ctx.enter_context(tc.tile_pool(name="psum", bufs=1, space="PSUM"))

        # --- PE pre-warm: ~4us of dummy matmuls unthrottles the HAM clock
        # gate (PE boots at 1.2 GHz; 3.4us of sustained activity -> 2.4 GHz).
        # Uses a preloaded const AP so nothing gates the first matmul.
        warm_c = nc.const_aps.tensor(0.0, (128, 64), f32)
        pwarm = psum.tile([64, 64], f32, tag="misc", bufs=1, name="pwarm")
        for _ in range(22):
            mm(pwarm[:], warm_c, warm_c, start=True, stop=True,
               skip_group_check=True)

        # --- loads. cc first (first tanh needs it early); panel-0 weights
        # per-k interleaved with x so the k-outer wave streams as data
        # lands; everything later is batched into few large DMAs.
        cc_sb = consts.tile([128, 55], f32, name="cc_sb")
        nc.sync.dma_start(cc_sb[:], cc[:])

        xt_sb = consts.tile([128, KC * BC], mmdt, name="xt_sb")
        wp0 = wpool.tile([128, KC * 1024], mmdt, tag="wpanel", name="wp0")
        for k in range(KC):
            nc.sync.dma_start(
                xt_sb[:, k * BC:k * BC + BN], xt[k * 128:(k + 1) * 128, 0:BN]
            )
            nc.sync.dma_start(
                wp0[:, k * 1024:k * 1024 + 512],
                lwh[0, :, k * 1024:k * 1024 + 512],
            )
        for k in range(KC):
            nc.sync.dma_start(
                wp0[:, k * 1024 + 512:(k + 1) * 1024],
                lwh[0, :, k * 1024 + 512:(k + 1) * 1024],
            )
        # x second halves: one 3D DMA
        nc.sync.dma_start(
            xt_sb[:].rearrange("p (k c) -> p k c", c=BC)[:, :, BN:BC],
            xt[:, BN:BC].rearrange("(k p) c -> p k c", p=128),
        )
        cw_sb = consts.tile([128, I * 2 * 128], mmdt, name="cw_sb")
        nc.sync.dma_start(cw_sb[:], cw[:])
        wps = {0: wp0}
        rw2_sb = consts.tile([128, NPANEL * 128 + 1], mmdt, name="rw2_sb")
        for q in (1, 2, 3):
            wps[q] = wpool.tile([128, KC * 1024], mmdt, tag="wpanel", name=f"wp{q}")
            nc.sync.dma_start(wps[q][:], lwh[q])
            if q == 1:
                nc.sync.dma_start(rw2_sb[:], rw2[:])

        # scalar-engine warm: force the tanh ACT table load during the DMA
        # preamble instead of on the first real activation
        act_warm = work.tile([1, 1], f32, tag="actw", bufs=1, name="act_warm")
        nc.scalar.activation(act_warm[:], pwarm[0:1, 0:1], Tanh)

        # leaf predicts: node i at cols i*BC (+bn*BN); flushed per panel
        lp_sb = keep.tile([8, I * BC], f32, name="lp_sb")
        intp_sb = keep.tile([16, BC], f32, name="intp_sb")
        rootp_sb = keep.tile([1, BC], f32, name="rootp_sb")

        inth = {}      # (panel, bn) -> [128, BN] tile: nodes 4p..4p+3 h^T
        pending = []   # deferred emissions injected into the next stream
        prc1 = psum.tile([128, BN], f32, tag="prcinc", bufs=1, name="prc1")

        def emit_pending():
            for f in pending:
                f()
            pending.clear()

        def leaf_mm(wp, gl, k, bn, pg):
            mm(
                pg[:],
                wp[:, k * 1024 + gl * 128:k * 1024 + (gl + 1) * 128],
                xt_sb[:, k * BC + bn * BN:k * BC + bn * BN + BN],
                start=(k == 0),
                stop=(k == KC - 1),
            )

        def leaf_tanh(p, gl, bn, pg):
            lh = work.tile([128, BN], mmdt, tag="lh", name=f"lh{p}{bn}{gl}")
            nc.scalar.activation(
                lh[:], pg[:], Tanh, bias=cc_sb[:, GPP * p + gl:GPP * p + gl + 1]
            )
            return lh

        def comb_mm(p, il, j, lh, pcomb):
            """Fused internal-trans + leaf-predict matmul.

            pcomb rows 0:32 accumulate node (4p+il)'s hidden
            pre-activation over its two child groups; rows 32:40 pick up
            the group's 4 leaf predict dots via the block-diagonal
            columns (zeros elsewhere)."""
            i = 4 * p + il
            mm(
                pcomb[:],
                cw_sb[:, (2 * i + j) * 128:(2 * i + j + 1) * 128],
                lh[:],
                start=(j == 0),
                stop=(j == 1),
                skip_group_check=True,
            )

        def comb_post(p, il, bn, ith, pcomb):
            i = 4 * p + il
            nc.scalar.activation(
                ith[32 * il:32 * il + 32, :],
                pcomb[0:32, :],
                Tanh,
                bias=cc_sb[32 * il:32 * il + 32, 32 + p:33 + p],
            )
            nc.scalar.activation(
                lp_sb[:, i * BC + bn * BN:i * BC + bn * BN + BN],
                pcomb[32:40, :], Tanh, bias=cc_sb[0:8, 36 + i:37 + i],
            )

        def flush_lp(p):
            nc.sync.dma_start(
                out[32 * p:32 * (p + 1), :].rearrange("(i v) c -> v i c", v=8),
                lp_sb[:, 4 * p * BC:4 * (p + 1) * BC].rearrange(
                    "v (i c) -> v i c", c=BC
                ),
            )

        def root_mm_inc(p, ith):
            """One panel's contribution to the bn=1 root/int-predict
            contraction, accumulated across panels in a persistent bank."""
            mm(
                prc1[:],
                rw2_sb[:, 128 * p:128 * (p + 1)],
                ith[:],
                start=(p == 0),
                stop=(p == NPANEL - 1),
                skip_group_check=True,
            )

        def root_post(bn, prc):
            rh = work.tile([32, BN], mmdt, tag="rh", bufs=2, name=f"rh{bn}")
            nc.scalar.activation(rh[:], prc[0:32, :], Tanh,
                                 bias=cc_sb[0:32, 53:54])
            nc.scalar.activation(
                intp_sb[:, bn * BN:bn * BN + BN], prc[32:48, :], Tanh,
                bias=cc_sb[0:16, 52:53],
            )
            prp = psum.tile([1, BN], f32, tag="misc", bufs=1, name=f"prp{bn}")
            mm(prp[:], rw2_sb[0:32, NPANEL * 128:NPANEL * 128 + 1], rh[:],
               start=True, stop=True, skip_group_check=True)
            nc.scalar.activation(
                rootp_sb[0:1, bn * BN:bn * BN + BN], prp[:], Tanh,
                bias=cc_sb[0:1, 54:55],
            )

        # --- panel 0: pure leaf streams first (combs deferred until cw
        # lands), k-outer waves for bn=0 so matmuls chase the arriving
        # x/weight chunks
        lh_stash = {}
        for g0, cnt in ((0, 4), (4, 4)):
            pgs = [
                psum.tile([128, BN], f32, tag="pg", bufs=4, name=f"pgko{g0}{q}")
                for q in range(cnt)
            ]
            for k in range(KC):
                for q in range(cnt):
                    leaf_mm(wp0, g0 + q, k, 0, pgs[q])
            for q in range(cnt):
                lh_stash[(g0 + q, 0)] = leaf_tanh(0, g0 + q, 0, pgs[q])
        for gl in range(GPP):
            pg = psum.tile([128, BN], f32, tag="pg", bufs=4, name=f"pg0b{gl}")
            for k in range(KC):
                leaf_mm(wp0, gl, k, 1, pg)
            lh_stash[(gl, 1)] = leaf_tanh(0, gl, 1, pg)
        for bn in range(NBH):
            ith = keep.tile([128, BN], mmdt, tag=f"inth0{bn}", name=f"inth0{bn}")
            for il in range(4):
                pcomb = psum.tile([128, BN], f32, tag="pcomb", bufs=2,
                                  name=f"pc0{bn}{il}")
                for j in range(2):
                    comb_mm(0, il, j, lh_stash.pop((2 * il + j, bn)), pcomb)
                comb_post(0, il, bn, ith, pcomb)
            inth[(0, bn)] = ith
        pending.append(lambda: root_mm_inc(0, inth[(0, 1)]))
        pending.append(lambda: flush_lp(0))

        # --- panels 1..3 ---------------------------------------------------
        for p in range(1, NPANEL):
            wp = wps[p]
            for bn in range(NBH):
                ith = keep.tile([128, BN], mmdt, tag=f"inth{p}{bn}",
                                name=f"inth{p}{bn}")
                for il in range(4):
                    pcomb = psum.tile([128, BN], f32, tag="pcomb", bufs=2,
                                      name=f"pc{p}{bn}{il}")
                    for j in range(2):
                        gl = 2 * il + j
                        pg = psum.tile([128, BN], f32, tag="pg", bufs=4,
                                       name=f"pg{p}{bn}{gl}")
                        for k in range(KC):
                            leaf_mm(wp, gl, k, bn, pg)
                        lh = leaf_tanh(p, gl, bn, pg)
                        comb_mm(p, il, j, lh, pcomb)
                        if il == 0 and j == 1:
                            # inject deferred work (prev panel's root mm,
                            # lp flush, bn0 root chain) once this stream
                            # is 2 groups deep: the producer activations
                            # are long done, so no PE stall
                            emit_pending()
                    comb_post(p, il, bn, ith, pcomb)
                inth[(p, bn)] = ith

                if bn == 1:
                    if p < NPANEL - 1:
                        pending.append(
                            lambda p=p: root_mm_inc(p, inth[(p, 1)])
                        )
                        pending.append(lambda p=p: flush_lp(p))
                    else:
                        # final panel: bn0 root was deferred into this
                        # stream; close bn1 root here
                        root_mm_inc(p, inth[(p, 1)])
                        root_post(1, prc1)
                        flush_lp(p)
                        nc.sync.dma_start(out[L:L + I, :], intp_sb[:])
                        nc.sync.dma_start(out[L + I:NOUT, :], rootp_sb[:])
                elif bn == 0 and p == NPANEL - 1:
                    # bn0 root: lazy 4-matmul contraction, deferred into
                    # the panel3-bn1 stream
                    def bn0_root():
                        prc0 = psum.tile([128, BN], f32, tag="misc", bufs=1,
                                         name="prc0")
                        for q in range(NPANEL):
                            mm(
                                prc0[:],
                                rw2_sb[:, 128 * q:128 * (q + 1)],
                                inth[(q, 0)][:],
                                start=(q == 0),
                                stop=(q == NPANEL - 1),
                                skip_group_check=True,
                            )
                        root_post(0, prc0)
                    pending.append(bn0_root)

    nc.compile()
    return nc


def _pack_weights(inp):
    f = np.float32
    f16 = np.float16
    leaf_b = np.asarray(inp["leaf_b"], f)
    int_W = np.asarray(inp["int_W"], f)
    int_b = np.asarray(inp["int_b"], f)
    root_W = np.asarray(inp["root_W"], f)
    root_b = np.asarray(inp["root_b"], f)
    leaf_Wp = np.asarray(inp["leaf_Wp"], f)
    leaf_bp = np.asarray(inp["leaf_bp"], f)
    int_Wp = np.asarray(inp["int_Wp"], f)
    int_bp = np.asarray(inp["int_bp"], f)
    root_Wp = np.asarray(inp["root_Wp"], f)
    root_bp = np.asarray(inp["root_bp"], f)

    w = {}
    lw = np.asarray(inp["leaf_W"], f16).transpose(1, 0, 2).reshape(D, L * H)
    w["lwh"] = np.ascontiguousarray(
        lw.reshape(KC, 128, NPANEL, 1024).transpose(2, 1, 0, 3).reshape(
            NPANEL, 128, KC * 1024
        )
    )

    cw = np.zeros((128, I * 2 * 128), f16)
    for i in range(I):
        for j in range(2):
            base = (2 * i + j) * 128
            # int_W chunk j of node i: rows (c*32+h) = child (4j+c) hidden h
            cw[:, base:base + 32] = int_W[i, 128 * j:128 * (j + 1), :]
            for c in range(4):
                lv = 8 * i + 4 * j + c
                cw[c * 32:(c + 1) * 32, base + 32 + 4 * j + c] = leaf_Wp[lv, :, 0]
    w["cw"] = cw

    rw2 = np.zeros((128, NPANEL * 128 + 1), f16)
    for q in range(NPANEL):
        rw2[:, 128 * q:128 * q + 32] = root_W[128 * q:128 * (q + 1), :]
        for c in range(4):
            iv = 4 * q + c
            rw2[c * 32:(c + 1) * 32, 128 * q + 32 + 4 * q + c] = int_Wp[iv, :, 0]
    rw2[0:32, NPANEL * 128] = root_Wp[:, 0]
    w["rw2"] = rw2

    cc = np.zeros((128, 55), f)
    cc[:, 0:32] = leaf_b.reshape(32, 128).T       # leaf biases, col=h, part=leaf%...
    cc[:, 32:36] = int_b.reshape(4, 128).T
    cc[0:8, 36:52] = leaf_bp.reshape(16, 8).T
    cc[0:16, 52] = int_bp[:, 0]
    cc[0:32, 53] = root_b
    cc[0, 54] = root_bp[0]
    w["cc"] = cc
    return w


def kernel(**inputs):
    from concourse.bass_utils import run_bass_kernel_spmd

    nc = _CACHE.get("nc")
    if nc is None:
        nc = _CACHE["nc"] = _build_nc()

    x = np.asarray(inputs["x"], np.float32)
    w = _pack_weights(inputs)
    in_maps = []
    for c in range(NCORES):
        m = dict(w)
        m["xt"] = np.ascontiguousarray(x[c * BC:(c + 1) * BC, :].T.astype(np.float16))
        in_maps.append(m)

    res = run_bass_kernel_spmd(nc, in_maps, core_ids=list(range(NCORES)))
    _CACHE["last_res"] = res
    outs = [res.results[c]["out"] for c in range(NCORES)]
    full = np.concatenate([o[:, :, None] for o in outs], axis=1)  # [145, B, 1]
    return full.astype(np.float32)


# revision 13
# speedup vs baseline: 1.0918x; 1.0132x over previous
"""Trainium2 Bass kernel for nn_CombineNode_7395933684091 (gnn_message_passing).

Hierarchy: 128 leaf terms (each D=1024 -> H=32), 16 internal terms
(concat of 8 children hiddens, 256 -> 32), 1 root (concat of 16
internal hiddens, 512 -> 32); every term also has a 1-dim predict head.
All matmuls followed by tanh.

Strategy: data-parallel over batch across 8 cores (Bc = 1024 rows per
core), weights replicated. On-chip layout keeps hidden features on the
PARTITION axis ("h^T layout": tiles are [features, batch]), so every
level's contraction is a natural PE matmul and the child-concat is just
stacking partition tiles. x and all weights are repacked on the host so
every DMA is contiguous per partition.

Leaf level: 4 panels x 8 groups (4 leaves) x 8 k-chunk accumulated
[128,128]x[128,512] matmuls. The per-term predict heads ride along as
extra block-diagonal columns fused into the internal-level stationary
operand (cw) and the root-level stationary operand (rw2), so they cost
no extra PE streaming.

Matmul operands are float16: same PE stream rate as f32r (1 col/cycle)
but enables Fast Weight Load (fp32 disables FWL) so LDWEIGHTS hides
behind the matmul stream, and halves HBM + SBUF traffic. fp16's 10
mantissa bits keep the end-to-end max abs error ~1.6e-3 (vs 2e-2 gate).

All small stationaries (cw/rw2 blocks) are zero-padded to 128 columns:
narrow (col_grp) matmuls block the next LDWEIGHTS pull-ahead and cost
~+120ns each. The root contraction for the last batch half accumulates
incrementally (one matmul per panel, deferred into the next panel's
stream) so the end-of-kernel serial chain is short.
"""

import numpy as np

B, D, H = 8192, 1024, 32
L, I, CPI = 128, 16, 8
NCORES = 8
BC = B // NCORES      # 1024 batch rows per core
BN = 512              # batch tile width (one PSUM bank of f32)
NBH = BC // BN        # 2 batch halves
KC = D // 128         # 8 contraction chunks for the leaf level
NPANEL = 4            # leaf panels (8 groups of 4 leaves each)
GPP = 8               # groups per panel
NOUT = L + I + 1      # 145

MM_DT = "float16"

_CACHE = {}


def _build_nc():
    from contextlib import ExitStack

    import concourse.mybir as mybir
    import concourse.tile as tile
    from concourse import bacc

    f32 = mybir.dt.float32
    Tanh = mybir.ActivationFunctionType.Tanh
    mmdt = getattr(mybir.dt, MM_DT)

    nc = bacc.Bacc("TRN2", target_bir_lowering=False, debug=False)

    xt = nc.dram_tensor("xt", [D, BC], mmdt, kind="ExternalInput")
    # leaf weights, panel-major: lwh[p, pp, k*1024 + j] so each panel is
    # one contiguous [128, 8K] DMA (16KB/partition runs)
    lwh = nc.dram_tensor("lwh", [NPANEL, 128, KC * 1024], mmdt, kind="ExternalInput")
    # fused internal-trans + leaf-predict stationary: per (node i, chunk j)
    # a [128, 128] block: cols 0:32 int_W chunk, col 32+4j+c leaf Wp diag,
    # rest zero padding (full-width stationaries keep LDW pull-ahead alive)
    cw = nc.dram_tensor("cw", [128, I * 2 * 128], mmdt, kind="ExternalInput")
    # fused root-trans + int-predict stationary: per panel q a [128, 128]
    # block (cols 0:32 root_W chunk, 32:48 int Wp diag, rest zero); final
    # column 512 holds root_Wp in rows 0:32
    rw2 = nc.dram_tensor("rw2", [128, NPANEL * 128 + 1], mmdt, kind="ExternalInput")
    # all f32 per-partition bias constants in one tensor:
    # cols 0:32 leaf_b, 32:36 int_b, 36:52 leaf_bp (rows 0:8),
    # 52 int_bp (rows 0:16), 53 root_b (rows 0:32), 54 root_bp (row 0)
    cc = nc.dram_tensor("cc", [128, 55], f32, kind="ExternalInput")
    out = nc.dram_tensor("out", [NOUT, BC], f32, kind="ExternalOutput")

    mm = nc.tensor.matmul

    with tile.TileContext(nc) as tc, ExitStack() as ctx:
        consts = ctx.enter_context(tc.tile_pool(name="consts", bufs=1))
        wpool = ctx.enter_context(tc.tile_pool(name="wpool", bufs=4))
        work = ctx.enter_context(tc.tile_pool(name="work", bufs=18))
        keep = ctx.enter_context(tc.tile_pool(name="keep", bufs=1))
        psum = ctx.enter_context(tc.tile_pool(name="psum", bufs=1, space="PSUM"))

        # --- PE pre-warm: ~4us of dummy matmuls unthrottles the HAM clock
        # gate (PE boots at 1.2 GHz; 3.4us of sustained activity -> 2.4 GHz).
        # Uses a preloaded const AP so nothing gates the first matmul.
        warm_c = nc.const_aps.tensor(0.0, (128, 64), f32)
        pwarm = psum.tile([64, 64], f32, tag="misc", bufs=1, name="pwarm")
        for _ in range(16):
            mm(pwarm[:], warm_c, warm_c, start=True, stop=True,
               skip_group_check=True)

        # --- loads. x + biases ride the Scalar engine's HW DMA queue and
        # the weights ride Sync's: two descriptor generators run in
        # parallel, doubling early-phase arrival rate. cc goes first (the
        # first tanh needs it); panel-0 weights per-k interleaved with x
        # so the k-outer wave streams as data lands; later panels are
        # single large DMAs (16KB/partition contiguous runs).
        cc_sb = consts.tile([128, 55], f32, name="cc_sb")
        nc.scalar.dma_start(cc_sb[:], cc[:])

        xt_sb = consts.tile([128, KC * BC], mmdt, name="xt_sb")
        wp0 = wpool.tile([128, KC * 1024], mmdt, tag="wpanel", name="wp0")
        for k in range(KC):
            nc.scalar.dma_start(
                xt_sb[:, k * BC:k * BC + BN], xt[k * 128:(k + 1) * 128, 0:BN]
            )
            nc.sync.dma_start(
                wp0[:, k * 1024:k * 1024 + 512],
                lwh[0, :, k * 1024:k * 1024 + 512],
            )
        # x second halves: one 3D DMA (scalar queue, runs behind xt bn0)
        nc.scalar.dma_start(
            xt_sb[:].rearrange("p (k c) -> p k c", c=BC)[:, :, BN:BC],
            xt[:, BN:BC].rearrange("(k p) c -> p k c", p=128),
        )
        for k in range(KC):
            nc.sync.dma_start(
                wp0[:, k * 1024 + 512:(k + 1) * 1024],
                lwh[0, :, k * 1024 + 512:(k + 1) * 1024],
            )
        cw_sb = consts.tile([128, I * 2 * 128], mmdt, name="cw_sb")
        nc.sync.dma_start(cw_sb[:], cw[:])
        wps = {0: wp0}
        rw2_sb = consts.tile([128, NPANEL * 128 + 1], mmdt, name="rw2_sb")
        for q in (1, 2, 3):
            wps[q] = wpool.tile([128, KC * 1024], mmdt, tag="wpanel", name=f"wp{q}")
            nc.sync.dma_start(wps[q][:], lwh[q])
            if q == 1:
                nc.sync.dma_start(rw2_sb[:], rw2[:])

        # scalar-engine warm: force the tanh ACT table load during the DMA
        # preamble instead of on the first real activation
        act_warm = work.tile([1, 1], f32, tag="actw", bufs=1, name="act_warm")
        nc.scalar.activation(act_warm[:], pwarm[0:1, 0:1], Tanh)

        # leaf predicts: node i at cols i*BC (+bn*BN); flushed per panel
        lp_sb = keep.tile([8, I * BC], f32, name="lp_sb")
        intp_sb = keep.tile([16, BC], f32, name="intp_sb")
        rootp_sb = keep.tile([1, BC], f32, name="rootp_sb")

        inth = {}      # (panel, bn) -> [128, BN] tile: nodes 4p..4p+3 h^T
        pending = []   # deferred emissions injected into the next stream
        prc1 = psum.tile([128, BN], f32, tag="prcinc", bufs=1, name="prc1")

        def emit_pending():
            for f in pending:
                f()
            pending.clear()

        def leaf_mm(wp, gl, k, bn, pg):
            mm(
                pg[:],
                wp[:, k * 1024 + gl * 128:k * 1024 + (gl + 1) * 128],
                xt_sb[:, k * BC + bn * BN:k * BC + bn * BN + BN],
                start=(k == 0),
                stop=(k == KC - 1),
            )

        def leaf_tanh(p, gl, bn, pg):
            lh = work.tile([128, BN], mmdt, tag="lh", name=f"lh{p}{bn}{gl}")
            nc.scalar.activation(
                lh[:], pg[:], Tanh, bias=cc_sb[:, GPP * p + gl:GPP * p + gl + 1]
            )
            return lh

        def comb_mm(p, il, j, lh, pcomb):
            """Fused internal-trans + leaf-predict matmul.

            pcomb rows 0:32 accumulate node (4p+il)'s hidden
            pre-activation over its two child groups; rows 32:40 pick up
            the group's 4 leaf predict dots via the block-diagonal
            columns (zeros elsewhere)."""
            i = 4 * p + il
            mm(
                pcomb[:],
                cw_sb[:, (2 * i + j) * 128:(2 * i + j + 1) * 128],
                lh[:],
                start=(j == 0),
                stop=(j == 1),
                skip_group_check=True,
            )

        def comb_post(p, il, bn, ith, pcomb):
            i = 4 * p + il
            nc.scalar.activation(
                ith[32 * il:32 * il + 32, :],
                pcomb[0:32, :],
                Tanh,
                bias=cc_sb[32 * il:32 * il + 32, 32 + p:33 + p],
            )
            nc.scalar.activation(
                lp_sb[:, i * BC + bn * BN:i * BC + bn * BN + BN],
                pcomb[32:40, :], Tanh, bias=cc_sb[0:8, 36 + i:37 + i],
            )

        def flush_lp(p, bn=None):
            if bn is None:
                nc.sync.dma_start(
                    out[32 * p:32 * (p + 1), :].rearrange("(i v) c -> v i c", v=8),
                    lp_sb[:, 4 * p * BC:4 * (p + 1) * BC].rearrange(
                        "v (i c) -> v i c", c=BC
                    ),
                )
            else:
                nc.sync.dma_start(
                    out[32 * p:32 * (p + 1), bn * BN:bn * BN + BN].rearrange(
                        "(i v) c -> v i c", v=8
                    ),
                    lp_sb[:].rearrange("v (i c) -> v i c", c=BC)[
                        :, 4 * p:4 * (p + 1), bn * BN:bn * BN + BN
                    ],
                )

        def root_mm_inc(p, ith):
            """One panel's contribution to the bn=1 root/int-predict
            contraction, accumulated across panels in a persistent bank."""
            mm(
                prc1[:],
                rw2_sb[:, 128 * p:128 * (p + 1)],
                ith[:],
                start=(p == 0),
                stop=(p == NPANEL - 1),
                skip_group_check=True,
            )

        def root_post(bn, prc):
            rh = work.tile([32, BN], mmdt, tag="rh", bufs=2, name=f"rh{bn}")
            nc.scalar.activation(rh[:], prc[0:32, :], Tanh,
                                 bias=cc_sb[0:32, 53:54])
            nc.scalar.activation(
                intp_sb[:, bn * BN:bn * BN + BN], prc[32:48, :], Tanh,
                bias=cc_sb[0:16, 52:53],
            )
            prp = psum.tile([1, BN], f32, tag="misc", bufs=1, name=f"prp{bn}")
            mm(prp[:], rw2_sb[0:32, NPANEL * 128:NPANEL * 128 + 1], rh[:],
               start=True, stop=True, skip_group_check=True)
            nc.scalar.activation(
                rootp_sb[0:1, bn * BN:bn * BN + BN], prp[:], Tanh,
                bias=cc_sb[0:1, 54:55],
            )

        # --- panel 0: pure leaf streams first (combs deferred until cw
        # lands), k-outer waves for bn=0 so matmuls chase the arriving
        # x/weight chunks
        lh_stash = {}
        for g0, cnt in ((0, 4), (4, 4)):
            pgs = [
                psum.tile([128, BN], f32, tag="pg", bufs=4, name=f"pgko{g0}{q}")
                for q in range(cnt)
            ]
            for k in range(KC):
                for q in range(cnt):
                    leaf_mm(wp0, g0 + q, k, 0, pgs[q])
            for q in range(cnt):
                lh_stash[(g0 + q, 0)] = leaf_tanh(0, g0 + q, 0, pgs[q])
        for gl in range(GPP):
            pg = psum.tile([128, BN], f32, tag="pg", bufs=4, name=f"pg0b{gl}")
            for k in range(KC):
                leaf_mm(wp0, gl, k, 1, pg)
            lh_stash[(gl, 1)] = leaf_tanh(0, gl, 1, pg)
        for bn in range(NBH):
            ith = keep.tile([128, BN], mmdt, tag=f"inth0{bn}", name=f"inth0{bn}")
            for il in range(4):
                pcomb = psum.tile([128, BN], f32, tag="pcomb", bufs=2,
                                  name=f"pc0{bn}{il}")
                for j in range(2):
                    comb_mm(0, il, j, lh_stash.pop((2 * il + j, bn)), pcomb)
                comb_post(0, il, bn, ith, pcomb)
            inth[(0, bn)] = ith
        pending.append(lambda: root_mm_inc(0, inth[(0, 1)]))
        pending.append(lambda: flush_lp(0))

        # --- panels 1..3. Panel 3 runs bn=1 first so its root chain
        # (incremental prc1) closes while bn=0's leaf stream still runs;
        # bn=0's lazy root contraction emits 3 of its 4 matmuls early, so
        # the end-of-kernel serial chain is one matmul + activations.
        prc0 = {}

        def prc0_partial():
            prc0["t"] = psum.tile([128, BN], f32, tag="misc", bufs=1,
                                  name="prc0")
            for q in range(NPANEL - 1):
                mm(
                    prc0["t"][:],
                    rw2_sb[:, 128 * q:128 * (q + 1)],
                    inth[(q, 0)][:],
                    start=(q == 0),
                    stop=False,
                    skip_group_check=True,
                )

        for p in range(1, NPANEL):
            wp = wps[p]
            bns = (1, 0) if p == NPANEL - 1 else (0, 1)
            for bn in bns:
                ith = keep.tile([128, BN], mmdt, tag=f"inth{p}{bn}",
                                name=f"inth{p}{bn}")
                for il in range(4):
                    pcomb = psum.tile([128, BN], f32, tag="pcomb", bufs=2,
                                      name=f"pc{p}{bn}{il}")
                    for j in range(2):
                        gl = 2 * il + j
                        pg = psum.tile([128, BN], f32, tag="pg", bufs=4,
                                       name=f"pg{p}{bn}{gl}")
                        for k in range(KC):
                            leaf_mm(wp, gl, k, bn, pg)
                        lh = leaf_tanh(p, gl, bn, pg)
                        comb_mm(p, il, j, lh, pcomb)
                        if il == 0 and j == 1:
                            # inject deferred work (prev panel's root mm,
                            # lp flush, root post chains) once this stream
                            # is 2 groups deep: the producer activations
                            # are long done, so no PE stall
                            emit_pending()
                    comb_post(p, il, bn, ith, pcomb)
                inth[(p, bn)] = ith

                if p < NPANEL - 1:
                    if bn == 1:
                        pending.append(
                            lambda p=p: root_mm_inc(p, inth[(p, 1)])
                        )
                        pending.append(lambda p=p: flush_lp(p))
                elif bn == 1:
                    # defer: close bn1 root + store its halves, then open
                    # bn0's root contraction (panels 0..2 ready now)
                    def close_bn1():
                        root_mm_inc(NPANEL - 1, inth[(NPANEL - 1, 1)])
                        root_post(1, prc1)
                        flush_lp(NPANEL - 1, 1)
                        nc.sync.dma_start(
                            out[L:L + I, BN:BC], intp_sb[:, BN:BC]
                        )
                    pending.append(close_bn1)
                    pending.append(prc0_partial)
                else:
                    # end of kernel: one matmul + act chain + small stores
                    mm(
                        prc0["t"][:],
                        rw2_sb[:, 128 * (NPANEL - 1):128 * NPANEL],
                        ith[:],
                        start=False,
                        stop=True,
                        skip_group_check=True,
                    )
                    root_post(0, prc0["t"])
                    flush_lp(NPANEL - 1, 0)
                    nc.sync.dma_start(out[L:L + I, 0:BN], intp_sb[:, 0:BN])
                    nc.sync.dma_start(out[L + I:NOUT, :], rootp_sb[:])

    nc.compile()
    return nc


def _pack_weights(inp):
    f = np.float32
    f16 = np.float16
    leaf_b = np.asarray(inp["leaf_b"], f)
    int_W = np.asarray(inp["int_W"], f)
    int_b = np.asarray(inp["int_b"], f)
    root_W = np.asarray(inp["root_W"], f)
    root_b = np.asarray(inp["root_b"], f)
    leaf_Wp = np.asarray(inp["leaf_Wp"], f)
    leaf_bp = np.asarray(inp["leaf_bp"], f)
    int_Wp = np.asarray(inp["int_Wp"], f)
    int_bp = np.asarray(inp["int_bp"], f)
    root_Wp = np.asarray(inp["root_Wp"], f)
    root_bp = np.asarray(inp["root_bp"], f)

    w = {}
    lw = np.asarray(inp["leaf_W"], f16).transpose(1, 0, 2).reshape(D, L * H)
    w["lwh"] = np.ascontiguousarray(
        lw.reshape(KC, 128, NPANEL, 1024).transpose(2, 1, 0, 3).reshape(
            NPANEL, 128, KC * 1024
        )
    )

    cw = np.zeros((128, I * 2 * 128), f16)
    for i in range(I):
        for j in range(2):
            base = (2 * i + j) * 128
            # int_W chunk j of node i: rows (c*32+h) = child (4j+c) hidden h
            cw[:, base:base + 32] = int_W[i, 128 * j:128 * (j + 1), :]
            for c in range(4):
                lv = 8 * i + 4 * j + c
                cw[c * 32:(c + 1) * 32, base + 32 + 4 * j + c] = leaf_Wp[lv, :, 0]
    w["cw"] = cw

    rw2 = np.zeros((128, NPANEL * 128 + 1), f16)
    for q in range(NPANEL):
        rw2[:, 128 * q:128 * q + 32] = root_W[128 * q:128 * (q + 1), :]
        for c in range(4):
            iv = 4 * q + c
            rw2[c * 32:(c + 1) * 32, 128 * q + 32 + 4 * q + c] = int_Wp[iv, :, 0]
    rw2[0:32, NPANEL * 128] = root_Wp[:, 0]
    w["rw2"] = rw2

    cc = np.zeros((128, 55), f)
    cc[:, 0:32] = leaf_b.reshape(32, 128).T       # leaf biases, col=h, part=leaf%...
    cc[:, 32:36] = int_b.reshape(4, 128).T
    cc[0:8, 36:52] = leaf_bp.reshape(16, 8).T
    cc[0:16, 52] = int_bp[:, 0]
    cc[0:32, 53] = root_b
    cc[0, 54] = root_bp[0]
    w["cc"] = cc
    return w


def kernel(**inputs):
    from concourse.bass_utils import run_bass_kernel_spmd

    nc = _CACHE.get("nc")
    if nc is None:
        nc = _CACHE["nc"] = _build_nc()

    x = np.asarray(inputs["x"], np.float32)
    w = _pack_weights(inputs)
    in_maps = []
    for c in range(NCORES):
        m = dict(w)
        m["xt"] = np.ascontiguousarray(x[c * BC:(c + 1) * BC, :].T.astype(np.float16))
        in_maps.append(m)

    res = run_bass_kernel_spmd(nc, in_maps, core_ids=list(range(NCORES)))
    _CACHE["last_res"] = res
    outs = [res.results[c]["out"] for c in range(NCORES)]
    full = np.concatenate([o[:, :, None] for o in outs], axis=1)  # [145, B, 1]
    return full.astype(np.float32)
